# revision 6
# baseline (speedup 1.0000x reference)
"""SMOTE.generate kernel for 8 TRN2 NeuronCores (Bass/Tile).

Problem: X [8192, 512] f32 -> pairwise sq-dists -> per-row 4 nearest
non-self neighbors -> pick by nn_choice -> synth = X + gaps*(X[sel]-X).
Output [32768, 512] f32.

Strategy (data-parallel over rows, 1024 rows/core):
  - s[r, c] = 2*x_r . x_c - |x_c|^2  has the same per-row ordering as
    -dist (per-row constant |x_r|^2 dropped; sqrt monotone).  Self is
    always the row max (|x_r|^2 vs ~ -|x_c|^2), matching the reference's
    top-1-is-self behavior.
  - GEMM on TensorE in fp32r (bf16-pair datapath, 4x faster than fp32) or
    bf16x3 (exact hi/lo split) / fp32 fallbacks; -|x_c|^2 enters as a
    rank-3 bf16 matmul (ones x [hi;lo;lo2] split of -sq).
  - Per 128-row block: DVE max8 + find_index8 over each 4096-col half,
    merge the 16 candidates, one-hot select by nn_choice, indirect-DMA
    gather X[sel], interpolate exactly in fp32.
"""
import os
import sys

import numpy as np

sys.path.insert(0, "/opt/trn_rl_repo")

T, D, N, KNN = 8192, 512, 4, 5
NCORES = 8
R = T // NCORES          # 1024 rows per core
P = 128
RB = R // P              # 8 row blocks per core
HALVES = 2
CH = T // HALVES         # 4096 columns per half
NB = 512                 # matmul free dim (one PSUM bank of fp32)
CB = CH // NB            # 8 col blocks per half
KC = D // P              # 4 contraction chunks of 128
DA = 528                 # gather row: x (512) | -sq (1) | pad; 64B-aligned rows

MODE = os.environ.get("SMOTE_MODE", "v2")  # v2 | bf16x3 | fp32r | fp32r_rr | fp32

_cache = {}


def _build_v2(v2dt="bf16", use_ttr=True, v2sdt="bf16"):
    """Single-pass low-precision GEMM shortlist + exact fp32 re-rank.

    s = 2*x_r.x_c - |x_c|^2 computed once in fp16 (1 cyc/row on PE, 3x
    cheaper than bf16x3).  PSUM is cast-copied to fp16 SBUF (+512 shift
    keeps values small for finer quantization).  DVE max8/find_index8
    gives an 8-wide shortlist per row (slot 0 is always self).  The 7
    non-self candidates are gathered in fp32 and re-ranked exactly with
    fused mul+reduce dot products, which restores the reference's fp32
    ordering (host sim: 0/32768 rows differ).
    """
    import concourse.bass as bass
    import concourse.bacc as bacc
    import concourse.mybir as mybir
    import concourse.tile as tile

    dt = mybir.dt
    AF = mybir.ActivationFunctionType
    ALU = mybir.AluOpType
    nc = bacc.Bacc("TRN2", target_bir_lowering=False, debug=False)

    mmdt = dt.float16 if v2dt == "fp16" else dt.bfloat16
    sdt = {"fp16": dt.float16, "bf16": dt.bfloat16, "fp32": dt.float32}[v2sdt]
    sbufs = 1 if v2sdt == "fp32" else 2  # fp32 s is 32KB/partition
    NCAND = 7  # non-self shortlist slots 1..7

    XTH = nc.dram_tensor("XTH", [D, T], mmdt, kind="ExternalInput").ap()
    XLT2H = nc.dram_tensor("XLT2H", [D, R], mmdt, kind="ExternalInput").ap()
    NEG3 = nc.dram_tensor("NEG3", [3, T], mmdt, kind="ExternalInput").ap()
    ONES3 = nc.dram_tensor("ONES3", [3, P], mmdt, kind="ExternalInput").ap()
    XAUG = nc.dram_tensor("XAUG", [T, DA], dt.float32, kind="ExternalInput").ap()
    XB2A = nc.dram_tensor("XB2A", [R, DA], dt.float32, kind="ExternalInput").ap()
    X = nc.dram_tensor("X", [T, D], dt.float32, kind="ExternalInput").ap()
    XL = nc.dram_tensor("XL", [R, D], dt.float32, kind="ExternalInput").ap()
    GAPS = nc.dram_tensor("GAPS", [R, N], dt.float32, kind="ExternalInput").ap()
    NCHF = nc.dram_tensor("NCHF", [R, N], dt.float32, kind="ExternalInput").ap()
    OUT = nc.dram_tensor("OUT", [R * N, D], dt.float32, kind="ExternalOutput").ap()
    OUT3 = OUT.rearrange("(r n) d -> r n d", n=N)

    with tile.TileContext(nc) as tc:
        with (
            tc.tile_pool(name="const", bufs=1) as const,
            tc.tile_pool(name="wk", bufs=2) as wk,
            tc.tile_pool(name="io", bufs=2) as io,
            tc.tile_pool(name="ps", bufs=2, space="PSUM") as ps,
        ):
            # ---- resident operands: X^T fp16 in 4x4 chunks, local 2X^T ----
            CCH = 2048
            NG = T // CCH
            xlt = [const.tile([P, R], mmdt, name=f"xlt{k}") for k in range(KC)]
            xt = [[const.tile([P, CCH], mmdt, name=f"xt{k}_{g}") for g in range(NG)]
                  for k in range(KC)]
            for k in range(KC):
                nc.sync.dma_start(xlt[k][:], XLT2H[k * P:(k + 1) * P, :])
            for k in range(KC):
                nc.sync.dma_start(xt[k][0][:], XTH[k * P:(k + 1) * P, 0:CCH])
            neg3 = const.tile([3, T], mmdt)
            ones3 = const.tile([3, P], mmdt)
            nc.sync.dma_start(neg3[:], NEG3[:])
            nc.sync.dma_start(ones3[:], ONES3[:])
            for g in range(1, NG):
                for k in range(KC):
                    nc.sync.dma_start(xt[k][g][:], XTH[k * P:(k + 1) * P, g * CCH:(g + 1) * CCH])

            for rb in range(RB):
                m0 = rb * P
                s16 = wk.tile([P, T], sdt, name=f"s16_{rb}", tag="s16", bufs=sbufs)
                for pg in range(NG):
                    pt = ps.tile([P, CCH], dt.float32, name=f"pt_{rb}_{pg}", tag="pt")
                    for cbi in range(CCH // NB):
                        gb = cbi * NB
                        b0 = pg * CCH + gb
                        for k in range(KC):
                            nc.tensor.matmul(pt[:, gb:gb + NB], lhsT=xlt[k][:, m0:m0 + P],
                                             rhs=xt[k][pg][:, gb:gb + NB],
                                             start=(k == 0), stop=False)
                        nc.tensor.matmul(pt[:, gb:gb + NB], lhsT=ones3[:, :],
                                         rhs=neg3[:, b0:b0 + NB], start=False, stop=True)
                    # cast-copy to fp16 with +512 shift (finer fp16 quantization)
                    nc.scalar.activation(s16[:, pg * CCH:(pg + 1) * CCH], pt[:],
                                         AF.Copy, bias=512.0, scale=1.0)

                # ---- top-8 shortlist (slot 0 = self) ----
                vals8 = wk.tile([P, 8], sdt, name=f"v8_{rb}", tag="v8")
                idxu = wk.tile([P, 8], dt.uint32, name=f"iu_{rb}", tag="iu")
                nc.vector.max(out=vals8[:], in_=s16[:])
                nc.vector.max_index(out=idxu[:], in_max=vals8[:], in_values=s16[:])

                # ---- gather the 7 non-self candidates (x | -sq) in fp32 ----
                xg = io.tile([P, NCAND * DA], dt.float32, name=f"xg_{rb}", tag="xg")
                for j in range(NCAND):
                    nc.gpsimd.indirect_dma_start(
                        out=xg[:, j * DA:(j + 1) * DA], out_offset=None, in_=XAUG[:],
                        in_offset=bass.IndirectOffsetOnAxis(ap=idxu[:, j + 1:j + 2], axis=0))

                # ---- exact re-rank: sex[:, j] = 2 x_r . x_c - |x_c|^2 in fp32 ----
                xb2a = io.tile([P, DA], dt.float32, name=f"xb2a_{rb}", tag="xb2a")
                nc.sync.dma_start(xb2a[:], XB2A[m0:m0 + P, :])
                sex = wk.tile([P, 8], dt.float32, name=f"sex_{rb}", tag="sex")
                nc.vector.memset(sex[:], -3.0e38)
                for j in range(NCAND):
                    scr = wk.tile([P, DA], dt.float32, name=f"scr_{rb}_{j}", tag="scr")
                    if use_ttr:
                        nc.vector.tensor_tensor_reduce(
                            out=scr[:], in0=xg[:, j * DA:(j + 1) * DA], in1=xb2a[:],
                            scale=1.0, scalar=0.0, op0=ALU.mult, op1=ALU.add,
                            accum_out=sex[:, j:j + 1])
                    else:
                        nc.vector.tensor_mul(scr[:], xg[:, j * DA:(j + 1) * DA], xb2a[:])
                        nc.vector.tensor_reduce(out=sex[:, j:j + 1], in_=scr[:],
                                                axis=mybir.AxisListType.X,
                                                op=ALU.add)

                # sort the 7 exact values; p2[r, t] = slot (0..6) of rank-t
                v2 = wk.tile([P, 8], dt.float32, name=f"v2_{rb}", tag="v2")
                p2u = wk.tile([P, 8], dt.uint32, name=f"p2u_{rb}", tag="p2u")
                p2f = wk.tile([P, 8], dt.float32, name=f"p2f_{rb}", tag="p2f")
                nc.vector.max(out=v2[:], in_=sex[:])
                nc.vector.max_index(out=p2u[:], in_max=v2[:], in_values=sex[:])
                nc.vector.tensor_copy(p2f[:], p2u[:])
                gidxf = wk.tile([P, 8], dt.float32, name=f"gx_{rb}", tag="gx")
                nc.vector.tensor_copy(gidxf[:], idxu[:])

                # fsel[r, n] = p2[r, nnc[r, n]]  (exact-rank -> shortlist slot)
                ncf = io.tile([P, N], dt.float32, name=f"ncf_{rb}", tag="ncf")
                nc.sync.dma_start(ncf[:], NCHF[m0:m0 + P, :])
                fsel = wk.tile([P, N], dt.float32, name=f"fs_{rb}", tag="fs")
                tmp4 = wk.tile([P, N], dt.float32, name=f"t4_{rb}", tag="t4")
                nc.vector.memset(fsel[:], 0.0)
                for j in range(N):
                    nc.vector.tensor_scalar(
                        out=tmp4[:], in0=ncf[:],
                        scalar1=float(j), scalar2=p2f[:, j:j + 1],
                        op0=ALU.is_equal, op1=ALU.mult)
                    nc.vector.tensor_add(fsel[:], fsel[:], tmp4[:])
                # sel[r, n] = idxu[r, 1 + fsel[r, n]]  (slot -> global column)
                self_f = wk.tile([P, N], dt.float32, name=f"sf_{rb}", tag="sf")
                tmp4b = wk.tile([P, N], dt.float32, name=f"t4b_{rb}", tag="t4b")
                nc.gpsimd.memset(self_f[:], 0.0)
                for j in range(NCAND):
                    nc.gpsimd.tensor_scalar(
                        out=tmp4b[:], in0=fsel[:],
                        scalar1=float(j), scalar2=gidxf[:, j + 1:j + 2],
                        op0=ALU.is_equal, op1=ALU.mult)
                    nc.gpsimd.tensor_add(self_f[:], self_f[:], tmp4b[:])
                selu = wk.tile([P, N], dt.uint32, name=f"su_{rb}", tag="su")
                nc.vector.tensor_copy(selu[:], self_f[:])

                # ---- gather selected rows, interpolate, store ----
                xb = io.tile([P, D], dt.float32, name=f"xb_{rb}", tag="xb")
                nc.sync.dma_start(xb[:], XL[m0:m0 + P, :])
                gaps_t = io.tile([P, N], dt.float32, name=f"gp_{rb}", tag="gp")
                nc.sync.dma_start(gaps_t[:], GAPS[m0:m0 + P, :])
                for n in range(N):
                    xs = io.tile([P, D], dt.float32, name=f"xs_{rb}_{n}", tag="xs", bufs=4)
                    nc.gpsimd.indirect_dma_start(
                        out=xs[:], out_offset=None, in_=X[:],
                        in_offset=bass.IndirectOffsetOnAxis(ap=selu[:, n:n + 1], axis=0))
                    df = io.tile([P, D], dt.float32, name=f"df_{rb}_{n}", tag="df", bufs=4)
                    nc.vector.tensor_sub(df[:], xs[:], xb[:])
                    nc.scalar.activation(df[:], df[:], AF.Copy, scale=gaps_t[:, n:n + 1])
                    ot = io.tile([P, D], dt.float32, name=f"ot_{rb}_{n}", tag="ot", bufs=4)
                    nc.gpsimd.tensor_add(ot[:], df[:], xb[:])
                    nc.sync.dma_start(OUT3[m0:m0 + P, n, :], ot[:])

    nc.compile()
    return nc


def _build(mode):
    import concourse.bass as bass
    import concourse.bacc as bacc
    import concourse.mybir as mybir
    import concourse.tile as tile

    dt = mybir.dt
    nc = bacc.Bacc("TRN2", target_bir_lowering=False, debug=False)

    if mode in ("fp32r", "fp32r_rr"):
        mmdt = dt.float32r
        XT = nc.dram_tensor("XT", [D, T], mmdt, kind="ExternalInput").ap()
        XLT2 = nc.dram_tensor("XLT2", [D, R], mmdt, kind="ExternalInput").ap()
    elif mode == "fp32":
        mmdt = dt.float32
        XT = nc.dram_tensor("XT", [D, T], mmdt, kind="ExternalInput").ap()
        XLT2 = nc.dram_tensor("XLT2", [D, R], mmdt, kind="ExternalInput").ap()
    elif mode == "bf16x3":
        mmdt = dt.bfloat16
        XTH = nc.dram_tensor("XTH", [D, T], mmdt, kind="ExternalInput").ap()
        XTL = nc.dram_tensor("XTL", [D, T], mmdt, kind="ExternalInput").ap()
        XLT2H = nc.dram_tensor("XLT2H", [D, R], mmdt, kind="ExternalInput").ap()
        XLT2L = nc.dram_tensor("XLT2L", [D, R], mmdt, kind="ExternalInput").ap()
    else:
        raise ValueError(mode)

    rr = mode == "fp32r_rr"
    if rr:
        XAUG = nc.dram_tensor("XAUG", [T, DA], dt.float32, kind="ExternalInput").ap()
        XB2A = nc.dram_tensor("XB2A", [R, DA], dt.float32, kind="ExternalInput").ap()
    NEG3 = nc.dram_tensor("NEG3", [3, T], dt.bfloat16, kind="ExternalInput").ap()
    ONES3 = nc.dram_tensor("ONES3", [3, P], dt.bfloat16, kind="ExternalInput").ap()
    X = nc.dram_tensor("X", [T, D], dt.float32, kind="ExternalInput").ap()
    XL = nc.dram_tensor("XL", [R, D], dt.float32, kind="ExternalInput").ap()
    GAPS = nc.dram_tensor("GAPS", [R, N], dt.float32, kind="ExternalInput").ap()
    NCHF = nc.dram_tensor("NCHF", [R, N], dt.float32, kind="ExternalInput").ap()
    OUT = nc.dram_tensor("OUT", [R * N, D], dt.float32, kind="ExternalOutput").ap()
    OUT3 = OUT.rearrange("(r n) d -> r n d", n=N)

    with tile.TileContext(nc) as tc:
        with (
            tc.tile_pool(name="const", bufs=1) as const,
            tc.tile_pool(name="sp", bufs=1) as sp,
            tc.tile_pool(name="wk", bufs=2) as wk,
            tc.tile_pool(name="io", bufs=2) as io,
            tc.tile_pool(name="ps", bufs=2, space="PSUM") as ps,
        ):
            # ---- resident operands (full X^T fits in SBUF) ----
            if mode == "bf16x3":
                CCH = 2048
                NG = T // CCH
                xlt_h = [const.tile([P, R], mmdt, name=f"xlth{k}") for k in range(KC)]
                xlt_l = [const.tile([P, R], mmdt, name=f"xltl{k}") for k in range(KC)]
                xt_h = [[const.tile([P, CCH], mmdt, name=f"xth{k}_{g}") for g in range(NG)]
                        for k in range(KC)]
                xt_l = [[const.tile([P, CCH], mmdt, name=f"xtl{k}_{g}") for g in range(NG)]
                        for k in range(KC)]
                for k in range(KC):
                    nc.sync.dma_start(xlt_h[k][:], XLT2H[k * P:(k + 1) * P, :])
                for k in range(KC):
                    nc.sync.dma_start(xt_h[k][0][:], XTH[k * P:(k + 1) * P, 0:CCH])
                for k in range(KC):
                    nc.sync.dma_start(xlt_l[k][:], XLT2L[k * P:(k + 1) * P, :])
                for k in range(KC):
                    nc.sync.dma_start(xt_l[k][0][:], XTL[k * P:(k + 1) * P, 0:CCH])
                for g in range(1, NG):
                    for k in range(KC):
                        nc.sync.dma_start(xt_h[k][g][:], XTH[k * P:(k + 1) * P, g * CCH:(g + 1) * CCH])
                        nc.sync.dma_start(xt_l[k][g][:], XTL[k * P:(k + 1) * P, g * CCH:(g + 1) * CCH])
            else:
                xlt = [const.tile([P, R], mmdt, name=f"xlt{k}") for k in range(KC)]
                xt = [const.tile([P, T], mmdt, name=f"xt{k}") for k in range(KC)]
                for k in range(KC):
                    nc.sync.dma_start(xt[k][:], XT[k * P:(k + 1) * P, :])
                    nc.sync.dma_start(xlt[k][:], XLT2[k * P:(k + 1) * P, :])
            neg3 = const.tile([3, T], dt.bfloat16)
            ones3 = const.tile([3, P], dt.bfloat16)
            nc.sync.dma_start(neg3[:], NEG3[:])
            nc.sync.dma_start(ones3[:], ONES3[:])

            PSB = 4  # col-blocks per PSUM tile (4 banks)
            for rb in range(RB):
                m0 = rb * P
                s = sp.tile([P, T], dt.float32, name=f"s_{rb}", tag="s")
                for pg in range(CB * HALVES // PSB):  # 4 groups of 4 col-blocks
                    pt = ps.tile([P, PSB * NB], dt.float32, name=f"pt_{rb}_{pg}", tag="pt")
                    for cbi in range(PSB):
                        b0 = (pg * PSB + cbi) * NB
                        o0 = cbi * NB
                        if mode == "bf16x3":
                            g, gb = b0 // 2048, b0 % 2048
                            for k in range(KC):
                                nc.tensor.matmul(pt[:, o0:o0 + NB], lhsT=xlt_h[k][:, m0:m0 + P],
                                                 rhs=xt_h[k][g][:, gb:gb + NB],
                                                 start=(k == 0), stop=False)
                            for k in range(KC):
                                nc.tensor.matmul(pt[:, o0:o0 + NB], lhsT=xlt_h[k][:, m0:m0 + P],
                                                 rhs=xt_l[k][g][:, gb:gb + NB],
                                                 start=False, stop=False)
                            for k in range(KC):
                                nc.tensor.matmul(pt[:, o0:o0 + NB], lhsT=xlt_l[k][:, m0:m0 + P],
                                                 rhs=xt_h[k][g][:, gb:gb + NB],
                                                 start=False, stop=False)
                        else:
                            for k in range(KC):
                                nc.tensor.matmul(pt[:, o0:o0 + NB], lhsT=xlt[k][:, m0:m0 + P],
                                                 rhs=xt[k][:, b0:b0 + NB],
                                                 start=(k == 0), stop=False)
                        # rank-3 bf16: adds -|x_c|^2 (hi+lo+lo2) exactly
                        nc.tensor.matmul(pt[:, o0:o0 + NB], lhsT=ones3[:, :],
                                         rhs=neg3[:, b0:b0 + NB], start=False, stop=True)
                    nc.scalar.copy(s[:, pg * PSB * NB:(pg + 1) * PSB * NB], pt[:])

                # ---- full-row top-8: values + global indices directly ----
                vals8 = wk.tile([P, 8], dt.float32, name=f"v8_{rb}", tag="v8")
                idxu = wk.tile([P, 8], dt.uint32, name=f"iu_{rb}", tag="iu")
                gidx = wk.tile([P, 8], dt.float32, name=f"gx_{rb}", tag="gx")
                nc.vector.max(out=vals8[:], in_=s[:])
                nc.vector.max_index(out=idxu[:], in_max=vals8[:], in_values=s[:])
                nc.vector.tensor_copy(gidx[:], idxu[:])

                xb = io.tile([P, D], dt.float32, name=f"xb{rb}", tag="xb", bufs=1)
                nc.gpsimd.dma_start(xb[:], XL[m0:m0 + P, :])

                if rr:
                    # ---- exact re-rank of the 8 fp32r-selected candidates ----
                    # gather XAUG rows (x_c | -|x_c|^2), recompute s exactly:
                    # s_ex[:, j] = reduce_add(2 * x_c * x_r, init=-sq_c)
                    sex = wk.tile([P, 8], dt.float32, name=f"sex{rb}", tag="sex")
                    xb2a = io.tile([P, DA], dt.float32, name=f"xb2a{rb}", tag="xb2a", bufs=1)
                    nc.sync.dma_start(xb2a[:], XB2A[m0:m0 + P, :])
                    for j in range(8):
                        xa = io.tile([P, DA], dt.float32, name=f"xa{rb}_{j}", tag="xa")
                        nc.gpsimd.indirect_dma_start(
                            out=xa[:], out_offset=None, in_=XAUG[:],
                            in_offset=bass.IndirectOffsetOnAxis(ap=idxu[:, j:j + 1], axis=0))
                        rrs = wk.tile([P, DA], dt.float32, name=f"rrs{rb}_{j}", tag="rrs")
                        nc.vector.tensor_mul(rrs[:], xa[:], xb2a[:])
                        nc.vector.tensor_reduce(out=sex[:, j:j + 1], in_=rrs[:],
                                                axis=mybir.AxisListType.X,
                                                op=mybir.AluOpType.add)
                    # sort the 8 exact values; map positions back to slots
                    v2 = wk.tile([P, 8], dt.float32, name=f"v2{rb}", tag="v2")
                    p2u = wk.tile([P, 8], dt.uint32, name=f"p2u{rb}", tag="p2u")
                    p2f = wk.tile([P, 8], dt.float32, name=f"p2f{rb}", tag="p2f")
                    nc.vector.max(out=v2[:], in_=sex[:])
                    nc.vector.max_index(out=p2u[:], in_max=v2[:], in_values=sex[:])
                    nc.vector.tensor_copy(p2f[:], p2u[:])

                # sel[r, n] = gidx[r, 1 + nn_choice[r, n]]
                ncf = io.tile([P, N], dt.float32, name=f"ncf{rb}", tag="ncf")
                nc.sync.dma_start(ncf[:], NCHF[m0:m0 + P, :])
                self_f = wk.tile([P, N], dt.float32, name=f"sf{rb}", tag="sf")
                tmp4 = wk.tile([P, N], dt.float32, name=f"t4{rb}", tag="t4")
                nc.vector.memset(self_f[:], 0.0)
                if rr:
                    # fsel[r, n] = p2[r, 1 + nnc[r, n]]  (rerank pos -> orig slot)
                    fsel = wk.tile([P, N], dt.float32, name=f"fs{rb}", tag="fs")
                    nc.vector.memset(fsel[:], 0.0)
                    for j in range(1, 5):
                        nc.vector.tensor_scalar(
                            out=tmp4[:], in0=ncf[:],
                            scalar1=float(j - 1), scalar2=p2f[:, j:j + 1],
                            op0=mybir.AluOpType.is_equal, op1=mybir.AluOpType.mult)
                        nc.vector.tensor_add(fsel[:], fsel[:], tmp4[:])
                    # sel[r, n] = gidx[r, fsel[r, n]]  (slot -> global col idx)
                    for p8 in range(1, 8):
                        nc.gpsimd.tensor_scalar(
                            out=tmp4[:], in0=fsel[:],
                            scalar1=float(p8), scalar2=gidx[:, p8:p8 + 1],
                            op0=mybir.AluOpType.is_equal, op1=mybir.AluOpType.mult)
                        nc.gpsimd.tensor_add(self_f[:], self_f[:], tmp4[:])
                else:
                    for j in range(1, 5):
                        nc.vector.tensor_scalar(
                            out=tmp4[:], in0=ncf[:],
                            scalar1=float(j - 1), scalar2=gidx[:, j:j + 1],
                            op0=mybir.AluOpType.is_equal, op1=mybir.AluOpType.mult)
                        nc.vector.tensor_add(self_f[:], self_f[:], tmp4[:])
                selu = wk.tile([P, N], dt.uint32, name=f"su{rb}", tag="su")
                nc.vector.tensor_copy(selu[:], self_f[:])

                gaps_t = io.tile([P, N], dt.float32, name=f"gp{rb}", tag="gp")
                nc.sync.dma_start(gaps_t[:], GAPS[m0:m0 + P, :])
                for n in range(N):
                    if rr:
                        xsw = io.tile([P, DA], dt.float32, name=f"xs{rb}_{n}", tag="xa")
                        nc.gpsimd.indirect_dma_start(
                            out=xsw[:], out_offset=None, in_=XAUG[:],
                            in_offset=bass.IndirectOffsetOnAxis(ap=selu[:, n:n + 1], axis=0))
                        xs = xsw[:, :D]
                    else:
                        xs = io.tile([P, D], dt.float32, name=f"xs{rb}_{n}", tag="xs", bufs=4)
                        nc.gpsimd.indirect_dma_start(
                            out=xs[:], out_offset=None, in_=X[:],
                            in_offset=bass.IndirectOffsetOnAxis(ap=selu[:, n:n + 1], axis=0))
                    df = io.tile([P, D], dt.float32, name=f"df{rb}_{n}", tag="df", bufs=2)
                    nc.gpsimd.tensor_sub(df[:], xs[:], xb[:])
                    nc.vector.tensor_scalar_mul(df[:], df[:], gaps_t[:, n:n + 1])
                    nc.gpsimd.tensor_add(df[:], df[:], xb[:])
                    nc.sync.dma_start(OUT3[m0:m0 + P, n, :], df[:])

    nc.compile()
    return nc


def _bf16(x):
    import ml_dtypes
    return x.astype(ml_dtypes.bfloat16)


def _pair_round(x):
    hi = _bf16(x).astype(np.float32)
    lo = _bf16(x - hi).astype(np.float32)
    return hi + lo


V2DT = os.environ.get("SMOTE_V2_DT", "bf16")
V2TTR = os.environ.get("SMOTE_V2_TTR", "1") == "1"
V2SDT = os.environ.get("SMOTE_V2_SDT", "bf16")


def _get_nc(mode):
    key = (mode, V2DT, V2TTR, V2SDT) if mode == "v2" else mode
    if key not in _cache:
        _cache[key] = _build_v2(V2DT, V2TTR, V2SDT) if mode == "v2" else _build(mode)
    return _cache[key]


def _kernel_v2(X, gaps, nnc):
    from concourse.bass_utils import run_bass_kernel_spmd

    nc = _get_nc("v2")

    sq = np.einsum("td,td->t", X, X, dtype=np.float32).astype(np.float32)
    negsq = -sq
    if V2DT == "fp16":
        f16 = lambda a: a.astype(np.float16)
    else:
        import ml_dtypes
        f16 = lambda a: a.astype(ml_dtypes.bfloat16)
    n1 = f16(negsq).astype(np.float32)
    n2 = f16(negsq - n1).astype(np.float32)
    n3 = f16(negsq - n1 - n2).astype(np.float32)
    NEG3 = np.ascontiguousarray(np.stack([f16(n1), f16(n2), f16(n3)]))
    ONES3 = np.ascontiguousarray(f16(np.ones((3, P), dtype=np.float32)))
    XTH = np.ascontiguousarray(f16(X.T))
    xaug = np.zeros((T, DA), dtype=np.float32)
    xaug[:, :D] = X
    xaug[:, D] = negsq
    common = dict(XTH=XTH, NEG3=NEG3, ONES3=ONES3, XAUG=xaug, X=X)

    in_maps = []
    for c in range(NCORES):
        r0 = c * R
        xl = X[r0:r0 + R]
        m = dict(common)
        m["XLT2H"] = np.ascontiguousarray(f16((2.0 * xl).T))
        m["XL"] = np.ascontiguousarray(xl)
        xb2a = np.zeros((R, DA), dtype=np.float32)
        xb2a[:, :D] = 2.0 * xl
        xb2a[:, D] = 1.0
        m["XB2A"] = xb2a
        m["GAPS"] = np.ascontiguousarray(gaps[r0:r0 + R])
        m["NCHF"] = np.ascontiguousarray(nnc[r0:r0 + R].astype(np.float32))
        in_maps.append(m)
    return nc, in_maps


def kernel(X, gaps, nn_choice, k, _want_results=False, _trace=False):
    X = np.ascontiguousarray(np.asarray(X, dtype=np.float32))
    gaps = np.ascontiguousarray(np.asarray(gaps, dtype=np.float32))
    nnc = np.asarray(nn_choice).astype(np.int64)
    assert int(k) == KNN and X.shape == (T, D) and gaps.shape == (T, N)

    from concourse.bass_utils import run_bass_kernel_spmd

    mode = MODE
    if mode == "v2":
        nc, in_maps = _kernel_v2(X, gaps, nnc)
        res = run_bass_kernel_spmd(nc, in_maps, core_ids=list(range(NCORES)), trace=_trace)
        out = np.concatenate([res.results[c]["OUT"] for c in range(NCORES)], axis=0)
        if _want_results:
            return out, res
        return out
    nc = _get_nc(mode)

    sq = np.einsum("td,td->t", X, X, dtype=np.float32).astype(np.float32)
    negsq = -sq
    n1 = _bf16(negsq).astype(np.float32)
    n2 = _bf16(negsq - n1).astype(np.float32)
    n3 = _bf16(negsq - n1 - n2).astype(np.float32)
    NEG3 = np.ascontiguousarray(np.stack([_bf16(n1), _bf16(n2), _bf16(n3)]))
    ONES3 = np.ascontiguousarray(np.ones((3, P), dtype=np.float32).astype(NEG3.dtype))
    XTc = np.ascontiguousarray(X.T)

    common = dict(NEG3=NEG3, ONES3=ONES3, X=X)
    if mode == "fp32r_rr":
        xaug = np.zeros((T, DA), dtype=np.float32)
        xaug[:, :D] = X
        xaug[:, D] = negsq
        common["XAUG"] = xaug
    if mode in ("fp32r", "fp32r_rr"):
        common["XT"] = np.ascontiguousarray(_pair_round(XTc))
    elif mode == "fp32":
        common["XT"] = XTc
    else:
        xth = _bf16(XTc)
        common["XTH"] = np.ascontiguousarray(xth)
        common["XTL"] = np.ascontiguousarray(_bf16(XTc - xth.astype(np.float32)))

    in_maps = []
    for c in range(NCORES):
        r0 = c * R
        xl = X[r0:r0 + R]
        xlt2 = np.ascontiguousarray((2.0 * xl).T)
        m = dict(common)
        if mode in ("fp32r", "fp32r_rr"):
            m["XLT2"] = np.ascontiguousarray(_pair_round(xlt2))
        elif mode == "fp32":
            m["XLT2"] = xlt2
        else:
            h = _bf16(xlt2)
            m["XLT2H"] = np.ascontiguousarray(h)
            m["XLT2L"] = np.ascontiguousarray(_bf16(xlt2 - h.astype(np.float32)))
        m["XL"] = np.ascontiguousarray(xl)
        if mode == "fp32r_rr":
            xb2a = np.zeros((R, DA), dtype=np.float32)
            xb2a[:, :D] = 2.0 * xl
            xb2a[:, D] = 1.0
            m["XB2A"] = xb2a
        m["GAPS"] = np.ascontiguousarray(gaps[r0:r0 + R])
        m["NCHF"] = np.ascontiguousarray(nnc[r0:r0 + R].astype(np.float32))
        in_maps.append(m)

    res = run_bass_kernel_spmd(nc, in_maps, core_ids=list(range(NCORES)), trace=_trace)
    out = np.concatenate([res.results[c]["OUT"] for c in range(NCORES)], axis=0)
    if _want_results:
        return out, res
    return out



# revision 10
# speedup vs baseline: 1.2478x; 1.2478x over previous
"""SMOTE.generate kernel for 8 TRN2 NeuronCores (Bass/Tile).

Problem: X [8192, 512] f32 -> pairwise sq-dists -> per-row 4 nearest
non-self neighbors -> pick by nn_choice -> synth = X + gaps*(X[sel]-X).
Output [32768, 512] f32.

Strategy (data-parallel over rows, 1024 rows/core):
  - s[r, c] = 2*x_r . x_c - |x_c|^2  has the same per-row ordering as
    -dist (per-row constant |x_r|^2 dropped; sqrt monotone).  Self is
    always the row max (|x_r|^2 vs ~ -|x_c|^2), matching the reference's
    top-1-is-self behavior.
  - GEMM on TensorE in fp32r (bf16-pair datapath, 4x faster than fp32) or
    bf16x3 (exact hi/lo split) / fp32 fallbacks; -|x_c|^2 enters as a
    rank-3 bf16 matmul (ones x [hi;lo;lo2] split of -sq).
  - Per 128-row block: DVE max8 + find_index8 over each 4096-col half,
    merge the 16 candidates, one-hot select by nn_choice, indirect-DMA
    gather X[sel], interpolate exactly in fp32.
"""
import os
import sys

import numpy as np

sys.path.insert(0, "/opt/trn_rl_repo")

T, D, N, KNN = 8192, 512, 4, 5
NCORES = 8
R = T // NCORES          # 1024 rows per core
P = 128
RB = R // P              # 8 row blocks per core
HALVES = 2
CH = T // HALVES         # 4096 columns per half
NB = 512                 # matmul free dim (one PSUM bank of fp32)
CB = CH // NB            # 8 col blocks per half
KC = D // P              # 4 contraction chunks of 128
DA = 528                 # gather row: x (512) | -sq (1) | pad; 64B-aligned rows

MODE = os.environ.get("SMOTE_MODE", "v2")  # v2 | bf16x3 | fp32r | fp32r_rr | fp32

_cache = {}


def _build_v2(v2dt="bf16", use_ttr=True, v2sdt="bf16", multigather=False):
    """Single-pass low-precision GEMM shortlist + exact fp32 re-rank.

    s = 2*x_r.x_c - |x_c|^2 computed once in fp16 (1 cyc/row on PE, 3x
    cheaper than bf16x3).  PSUM is cast-copied to fp16 SBUF (+512 shift
    keeps values small for finer quantization).  DVE max8/find_index8
    gives an 8-wide shortlist per row (slot 0 is always self).  The 7
    non-self candidates are gathered in fp32 and re-ranked exactly with
    fused mul+reduce dot products, which restores the reference's fp32
    ordering (host sim: 0/32768 rows differ).
    """
    import concourse.bass as bass
    import concourse.bacc as bacc
    import concourse.mybir as mybir
    import concourse.tile as tile

    dt = mybir.dt
    AF = mybir.ActivationFunctionType
    ALU = mybir.AluOpType
    nc = bacc.Bacc("TRN2", target_bir_lowering=False, debug=False)

    mmdt = dt.float16 if v2dt == "fp16" else dt.bfloat16
    sdt = {"fp16": dt.float16, "bf16": dt.bfloat16, "fp32": dt.float32}[v2sdt]
    sbufs = 1 if v2sdt == "fp32" else 2  # fp32 s is 32KB/partition
    NCAND = int(os.environ.get("SMOTE_V2_NC", "5"))  # non-self shortlist slots

    XTH = nc.dram_tensor("XTH", [D, T], mmdt, kind="ExternalInput").ap()
    XLT2H = nc.dram_tensor("XLT2H", [D, R], mmdt, kind="ExternalInput").ap()
    NEG3 = nc.dram_tensor("NEG3", [3, T], mmdt, kind="ExternalInput").ap()
    ONES3 = nc.dram_tensor("ONES3", [3, P], mmdt, kind="ExternalInput").ap()
    XAUG = nc.dram_tensor("XAUG", [T, DA], dt.float32, kind="ExternalInput").ap()
    XB2A = nc.dram_tensor("XB2A", [R, DA], dt.float32, kind="ExternalInput").ap()
    X = nc.dram_tensor("X", [T, D], dt.float32, kind="ExternalInput").ap()
    XL = nc.dram_tensor("XL", [R, D], dt.float32, kind="ExternalInput").ap()
    GAPS = nc.dram_tensor("GAPS", [R, N], dt.float32, kind="ExternalInput").ap()
    NCHF = nc.dram_tensor("NCHF", [R, N], dt.float32, kind="ExternalInput").ap()
    OUT = nc.dram_tensor("OUT", [R * N, D], dt.float32, kind="ExternalOutput").ap()
    OUT3 = OUT.rearrange("(r n) d -> r n d", n=N)

    with tile.TileContext(nc) as tc:
        with (
            tc.tile_pool(name="const", bufs=1) as const,
            tc.tile_pool(name="wk", bufs=2) as wk,
            tc.tile_pool(name="io", bufs=2) as io,
            tc.tile_pool(name="ps", bufs=2, space="PSUM") as ps,
        ):
            # ---- resident operands: X^T fp16 in 4x4 chunks, local 2X^T ----
            CCH = 2048
            NG = T // CCH
            xlt = [const.tile([P, R], mmdt, name=f"xlt{k}") for k in range(KC)]
            xt = [[const.tile([P, CCH], mmdt, name=f"xt{k}_{g}") for g in range(NG)]
                  for k in range(KC)]
            for k in range(KC):
                nc.sync.dma_start(xlt[k][:], XLT2H[k * P:(k + 1) * P, :])
            for k in range(KC):
                nc.sync.dma_start(xt[k][0][:], XTH[k * P:(k + 1) * P, 0:CCH])
            neg3 = const.tile([3, T], mmdt)
            ones3 = const.tile([3, P], mmdt)
            nc.sync.dma_start(neg3[:], NEG3[:])
            nc.sync.dma_start(ones3[:], ONES3[:])
            for g in range(1, NG):
                for k in range(KC):
                    nc.sync.dma_start(xt[k][g][:], XTH[k * P:(k + 1) * P, g * CCH:(g + 1) * CCH])

            for rb in range(RB):
                m0 = rb * P
                s16 = wk.tile([P, T], sdt, name=f"s16_{rb}", tag="s16", bufs=sbufs)
                for pg in range(NG):
                    pt = ps.tile([P, CCH], dt.float32, name=f"pt_{rb}_{pg}", tag="pt")
                    # k-outer: 4 consecutive matmuls share the same weights
                    for k in range(KC):
                        for cbi in range(CCH // NB):
                            gb = cbi * NB
                            nc.tensor.matmul(pt[:, gb:gb + NB], lhsT=xlt[k][:, m0:m0 + P],
                                             rhs=xt[k][pg][:, gb:gb + NB],
                                             start=(k == 0), stop=False,
                                             skip_group_check=True)
                    for cbi in range(CCH // NB):
                        gb = cbi * NB
                        b0 = pg * CCH + gb
                        nc.tensor.matmul(pt[:, gb:gb + NB], lhsT=ones3[:, :],
                                         rhs=neg3[:, b0:b0 + NB], start=False, stop=True,
                                         skip_group_check=True)
                    # cast-copy to 16-bit with +512 shift (finer quantization)
                    nc.scalar.activation(s16[:, pg * CCH:(pg + 1) * CCH], pt[:],
                                         AF.Copy, bias=512.0, scale=1.0)

                # ---- top-8 shortlist (slot 0 = self; slots 1..6 re-ranked) ----
                vals8 = wk.tile([P, 8], sdt, name=f"v8_{rb}", tag="v8")
                idxu = wk.tile([P, 8], dt.uint32, name=f"iu_{rb}", tag="iu")
                nc.vector.max(out=vals8[:], in_=s16[:])
                nc.vector.max_index(out=idxu[:], in_max=vals8[:], in_values=s16[:])

                # ---- gather the 6 best non-self candidates (x | -sq) in fp32 ----
                xg = io.tile([P, NCAND * DA], dt.float32, name=f"xg_{rb}", tag="xg")
                if multigather:
                    nc.gpsimd.indirect_dma_start(
                        out=xg[:], out_offset=None, in_=XAUG[:],
                        in_offset=bass.IndirectOffsetOnAxis(ap=idxu[:, 1:1 + NCAND], axis=0))
                else:
                    for j in range(NCAND):
                        nc.gpsimd.indirect_dma_start(
                            out=xg[:, j * DA:(j + 1) * DA], out_offset=None, in_=XAUG[:],
                            in_offset=bass.IndirectOffsetOnAxis(ap=idxu[:, j + 1:j + 2], axis=0))

                # ---- exact re-rank: sex[:, j] = 2 x_r . x_c - |x_c|^2 in fp32 ----
                xb2a = io.tile([P, DA], dt.float32, name=f"xb2a_{rb}", tag="xb2a")
                nc.sync.dma_start(xb2a[:], XB2A[m0:m0 + P, :])
                sex = wk.tile([P, 8], dt.float32, name=f"sex_{rb}", tag="sex")
                nc.vector.memset(sex[:], -3.0e38)
                NV = 2  # muls: j < NV on vector, rest on gpsimd; reduce on ACT accum
                for j in range(NCAND):
                    eng = nc.vector if j < NV else nc.gpsimd
                    scr = wk.tile([P, DA], dt.float32, name=f"scr_{rb}_{j}", tag="scrV" if j < NV else "scrG")
                    eng.tensor_mul(scr[:], xg[:, j * DA:(j + 1) * DA], xb2a[:])
                    scr2 = wk.tile([P, DA], dt.float32, name=f"scr2_{rb}_{j}", tag="scr2")
                    nc.scalar.activation(scr2[:], scr[:], AF.Copy,
                                         accum_out=sex[:, j:j + 1])

                # sort the 6 exact values; p2[r, t] = slot (0..5) of rank-t
                v2 = wk.tile([P, 8], dt.float32, name=f"v2_{rb}", tag="v2")
                p2u = wk.tile([P, 8], dt.uint32, name=f"p2u_{rb}", tag="p2u")
                p2f = wk.tile([P, 8], dt.float32, name=f"p2f_{rb}", tag="p2f")
                nc.vector.max(out=v2[:], in_=sex[:])
                nc.vector.max_index(out=p2u[:], in_max=v2[:], in_values=sex[:])
                nc.gpsimd.tensor_copy(p2f[:], p2u[:])
                gidxf = wk.tile([P, 8], dt.float32, name=f"gx_{rb}", tag="gx")
                nc.gpsimd.tensor_copy(gidxf[:], idxu[:])

                # fsel[r, n] = p2[r, nnc[r, n]]  (exact-rank -> shortlist slot)
                ncf = io.tile([P, N], dt.float32, name=f"ncf_{rb}", tag="ncf")
                nc.sync.dma_start(ncf[:], NCHF[m0:m0 + P, :])
                fsel = wk.tile([P, N], dt.float32, name=f"fs_{rb}", tag="fs")
                tmp4 = wk.tile([P, N], dt.float32, name=f"t4_{rb}", tag="t4")
                nc.vector.memset(fsel[:], 0.0)
                for j in range(N):
                    nc.vector.tensor_scalar(
                        out=tmp4[:], in0=ncf[:],
                        scalar1=float(j), scalar2=p2f[:, j:j + 1],
                        op0=ALU.is_equal, op1=ALU.mult)
                    nc.vector.tensor_add(fsel[:], fsel[:], tmp4[:])
                # sel[r, n] = idxu[r, 1 + fsel[r, n]]  (slot -> global column)
                self_f = wk.tile([P, N], dt.float32, name=f"sf_{rb}", tag="sf")
                tmp4b = wk.tile([P, N], dt.float32, name=f"t4b_{rb}", tag="t4b")
                nc.gpsimd.memset(self_f[:], 0.0)
                for j in range(NCAND):
                    nc.gpsimd.tensor_scalar(
                        out=tmp4b[:], in0=fsel[:],
                        scalar1=float(j), scalar2=gidxf[:, j + 1:j + 2],
                        op0=ALU.is_equal, op1=ALU.mult)
                    nc.gpsimd.tensor_add(self_f[:], self_f[:], tmp4b[:])
                selu = wk.tile([P, N], dt.uint32, name=f"su_{rb}", tag="su")
                nc.gpsimd.tensor_copy(selu[:], self_f[:])

                # ---- gather selected rows, interpolate, store ----
                # ot = g*xs + (1-g)*xb  (mul on ACT via per-partition scale)
                xb = io.tile([P, D], dt.float32, name=f"xb_{rb}", tag="xb")
                nc.sync.dma_start(xb[:], XL[m0:m0 + P, :])
                gaps_t = io.tile([P, N], dt.float32, name=f"gp_{rb}", tag="gp")
                nc.sync.dma_start(gaps_t[:], GAPS[m0:m0 + P, :])
                hfac = wk.tile([P, N], dt.float32, name=f"hf_{rb}", tag="hf")
                nc.gpsimd.tensor_scalar(out=hfac[:], in0=gaps_t[:], scalar1=-1.0,
                                        scalar2=1.0, op0=ALU.mult, op1=ALU.add)
                xs4 = io.tile([P, N * D], dt.float32, name=f"xs4_{rb}", tag="xs4")
                if multigather:
                    nc.gpsimd.indirect_dma_start(
                        out=xs4[:], out_offset=None, in_=X[:],
                        in_offset=bass.IndirectOffsetOnAxis(ap=selu[:, :], axis=0))
                for n in range(N):
                    if not multigather:
                        nc.gpsimd.indirect_dma_start(
                            out=xs4[:, n * D:(n + 1) * D], out_offset=None, in_=X[:],
                            in_offset=bass.IndirectOffsetOnAxis(ap=selu[:, n:n + 1], axis=0))
                    ht = io.tile([P, D], dt.float32, name=f"ht_{rb}_{n}", tag="ht", bufs=2)
                    nc.scalar.activation(ht[:], xb[:], AF.Copy, scale=hfac[:, n:n + 1])
                    df = io.tile([P, D], dt.float32, name=f"df_{rb}_{n}", tag="df", bufs=2)
                    nc.scalar.activation(df[:], xs4[:, n * D:(n + 1) * D], AF.Copy,
                                         scale=gaps_t[:, n:n + 1])
                    ot = io.tile([P, D], dt.float32, name=f"ot_{rb}_{n}", tag="ot", bufs=2)
                    nc.gpsimd.tensor_add(ot[:], df[:], ht[:])
                    nc.sync.dma_start(OUT3[m0:m0 + P, n, :], ot[:])

    nc.compile()
    return nc


def _bf16(x):
    import ml_dtypes
    return x.astype(ml_dtypes.bfloat16)


def _pair_round(x):
    hi = _bf16(x).astype(np.float32)
    lo = _bf16(x - hi).astype(np.float32)
    return hi + lo


V2DT = os.environ.get("SMOTE_V2_DT", "fp16")
V2TTR = os.environ.get("SMOTE_V2_TTR", "0") == "1"
V2SDT = os.environ.get("SMOTE_V2_SDT", "fp16")
V2MG = os.environ.get("SMOTE_V2_MG", "0") == "1"


def _get_nc(mode):
    key = (mode, V2DT, V2TTR, V2SDT, V2MG, os.environ.get("SMOTE_V2_NC", "5")) if mode == "v2" else mode
    if key not in _cache:
        _cache[key] = _build_v2(V2DT, V2TTR, V2SDT, V2MG) if mode == "v2" else _build(mode)
    return _cache[key]


def _kernel_v2(X, gaps, nnc):
    from concourse.bass_utils import run_bass_kernel_spmd

    nc = _get_nc("v2")

    sq = np.einsum("td,td->t", X, X, dtype=np.float32).astype(np.float32)
    negsq = -sq
    if V2DT == "fp16":
        f16 = lambda a: a.astype(np.float16)
    else:
        import ml_dtypes
        f16 = lambda a: a.astype(ml_dtypes.bfloat16)
    n1 = f16(negsq).astype(np.float32)
    n2 = f16(negsq - n1).astype(np.float32)
    n3 = f16(negsq - n1 - n2).astype(np.float32)
    NEG3 = np.ascontiguousarray(np.stack([f16(n1), f16(n2), f16(n3)]))
    ONES3 = np.ascontiguousarray(f16(np.ones((3, P), dtype=np.float32)))
    XTH = np.ascontiguousarray(f16(X.T))
    xaug = np.zeros((T, DA), dtype=np.float32)
    xaug[:, :D] = X
    xaug[:, D] = negsq
    common = dict(XTH=XTH, NEG3=NEG3, ONES3=ONES3, XAUG=xaug, X=X)

    in_maps = []
    for c in range(NCORES):
        r0 = c * R
        xl = X[r0:r0 + R]
        m = dict(common)
        m["XLT2H"] = np.ascontiguousarray(f16((2.0 * xl).T))
        m["XL"] = np.ascontiguousarray(xl)
        xb2a = np.zeros((R, DA), dtype=np.float32)
        xb2a[:, :D] = 2.0 * xl
        xb2a[:, D] = 1.0
        m["XB2A"] = xb2a
        m["GAPS"] = np.ascontiguousarray(gaps[r0:r0 + R])
        m["NCHF"] = np.ascontiguousarray(nnc[r0:r0 + R].astype(np.float32))
        in_maps.append(m)
    return nc, in_maps


def kernel(X, gaps, nn_choice, k, _want_results=False, _trace=False):
    X = np.ascontiguousarray(np.asarray(X, dtype=np.float32))
    gaps = np.ascontiguousarray(np.asarray(gaps, dtype=np.float32))
    nnc = np.asarray(nn_choice).astype(np.int64)
    assert int(k) == KNN and X.shape == (T, D) and gaps.shape == (T, N)

    from concourse.bass_utils import run_bass_kernel_spmd

    mode = MODE
    if mode == "v2":
        nc, in_maps = _kernel_v2(X, gaps, nnc)
        res = run_bass_kernel_spmd(nc, in_maps, core_ids=list(range(NCORES)), trace=_trace)
        out = np.concatenate([res.results[c]["OUT"] for c in range(NCORES)], axis=0)
        if _want_results:
            return out, res
        return out
    nc = _get_nc(mode)

    sq = np.einsum("td,td->t", X, X, dtype=np.float32).astype(np.float32)
    negsq = -sq
    n1 = _bf16(negsq).astype(np.float32)
    n2 = _bf16(negsq - n1).astype(np.float32)
    n3 = _bf16(negsq - n1 - n2).astype(np.float32)
    NEG3 = np.ascontiguousarray(np.stack([_bf16(n1), _bf16(n2), _bf16(n3)]))
    ONES3 = np.ascontiguousarray(np.ones((3, P), dtype=np.float32).astype(NEG3.dtype))
    XTc = np.ascontiguousarray(X.T)

    common = dict(NEG3=NEG3, ONES3=ONES3, X=X)
    if mode == "fp32r_rr":
        xaug = np.zeros((T, DA), dtype=np.float32)
        xaug[:, :D] = X
        xaug[:, D] = negsq
        common["XAUG"] = xaug
    if mode in ("fp32r", "fp32r_rr"):
        common["XT"] = np.ascontiguousarray(_pair_round(XTc))
    elif mode == "fp32":
        common["XT"] = XTc
    else:
        xth = _bf16(XTc)
        common["XTH"] = np.ascontiguousarray(xth)
        common["XTL"] = np.ascontiguousarray(_bf16(XTc - xth.astype(np.float32)))

    in_maps = []
    for c in range(NCORES):
        r0 = c * R
        xl = X[r0:r0 + R]
        xlt2 = np.ascontiguousarray((2.0 * xl).T)
        m = dict(common)
        if mode in ("fp32r", "fp32r_rr"):
            m["XLT2"] = np.ascontiguousarray(_pair_round(xlt2))
        elif mode == "fp32":
            m["XLT2"] = xlt2
        else:
            h = _bf16(xlt2)
            m["XLT2H"] = np.ascontiguousarray(h)
            m["XLT2L"] = np.ascontiguousarray(_bf16(xlt2 - h.astype(np.float32)))
        m["XL"] = np.ascontiguousarray(xl)
        if mode == "fp32r_rr":
            xb2a = np.zeros((R, DA), dtype=np.float32)
            xb2a[:, :D] = 2.0 * xl
            xb2a[:, D] = 1.0
            m["XB2A"] = xb2a
        m["GAPS"] = np.ascontiguousarray(gaps[r0:r0 + R])
        m["NCHF"] = np.ascontiguousarray(nnc[r0:r0 + R].astype(np.float32))
        in_maps.append(m)

    res = run_bass_kernel_spmd(nc, in_maps, core_ids=list(range(NCORES)), trace=_trace)
    out = np.concatenate([res.results[c]["OUT"] for c in range(NCORES)], axis=0)
    if _want_results:
        return out, res
    return out



# revision 13
# speedup vs baseline: 1.3466x; 1.0791x over previous
"""SMOTE.generate kernel for 8 TRN2 NeuronCores (Bass/Tile).

Problem: X [8192, 512] f32 -> pairwise sq-dists -> per-row 4 nearest
non-self neighbors -> pick by nn_choice -> synth = X + gaps*(X[sel]-X).
Output [32768, 512] f32.

Strategy (data-parallel over rows, 1024 rows/core):
  - s[r, c] = 2*x_r . x_c - |x_c|^2  has the same per-row ordering as
    -dist (per-row constant |x_r|^2 dropped; sqrt monotone).  Self is
    always the row max (|x_r|^2 vs ~ -|x_c|^2), matching the reference's
    top-1-is-self behavior.
  - GEMM on TensorE in fp32r (bf16-pair datapath, 4x faster than fp32) or
    bf16x3 (exact hi/lo split) / fp32 fallbacks; -|x_c|^2 enters as a
    rank-3 bf16 matmul (ones x [hi;lo;lo2] split of -sq).
  - Per 128-row block: DVE max8 + find_index8 over each 4096-col half,
    merge the 16 candidates, one-hot select by nn_choice, indirect-DMA
    gather X[sel], interpolate exactly in fp32.
"""
import os
import sys

import numpy as np

sys.path.insert(0, "/opt/trn_rl_repo")

T, D, N, KNN = 8192, 512, 4, 5
NCORES = 8
R = T // NCORES          # 1024 rows per core
P = 128
RB = R // P              # 8 row blocks per core
HALVES = 2
CH = T // HALVES         # 4096 columns per half
NB = 512                 # matmul free dim (one PSUM bank of fp32)
CB = CH // NB            # 8 col blocks per half
KC = D // P              # 4 contraction chunks of 128
DA = 528                 # gather row: x (512) | -sq (1) | pad; 64B-aligned rows

MODE = os.environ.get("SMOTE_MODE", "v2")  # v2 | bf16x3 | fp32r | fp32r_rr | fp32

_cache = {}


def _build_v2(v2dt="bf16", use_ttr=True, v2sdt="bf16", multigather=False):
    """Single-pass low-precision GEMM shortlist + exact fp32 re-rank.

    s = 2*x_r.x_c - |x_c|^2 computed once in fp16 (1 cyc/row on PE, 3x
    cheaper than bf16x3).  PSUM is cast-copied to fp16 SBUF (+512 shift
    keeps values small for finer quantization).  DVE max8/find_index8
    gives an 8-wide shortlist per row (slot 0 is always self).  The 7
    non-self candidates are gathered in fp32 and re-ranked exactly with
    fused mul+reduce dot products, which restores the reference's fp32
    ordering (host sim: 0/32768 rows differ).
    """
    import concourse.bass as bass
    import concourse.bacc as bacc
    import concourse.mybir as mybir
    import concourse.tile as tile

    dt = mybir.dt
    AF = mybir.ActivationFunctionType
    ALU = mybir.AluOpType
    nc = bacc.Bacc("TRN2", target_bir_lowering=False, debug=False)

    mmdt = dt.float16 if v2dt == "fp16" else dt.bfloat16
    sdt = {"fp16": dt.float16, "bf16": dt.bfloat16, "fp32": dt.float32}[v2sdt]
    sbufs = 1 if v2sdt == "fp32" else 2  # fp32 s is 32KB/partition
    NCAND = int(os.environ.get("SMOTE_V2_NC", "5"))  # non-self shortlist slots

    XTH = nc.dram_tensor("XTH", [D, T], mmdt, kind="ExternalInput").ap()
    XLT2H = nc.dram_tensor("XLT2H", [D, R], mmdt, kind="ExternalInput").ap()
    NEG3 = nc.dram_tensor("NEG3", [3, T], mmdt, kind="ExternalInput").ap()
    ONES3 = nc.dram_tensor("ONES3", [3, P], mmdt, kind="ExternalInput").ap()
    XAUG = nc.dram_tensor("XAUG", [T, DA], dt.float32, kind="ExternalInput").ap()
    XB2A = nc.dram_tensor("XB2A", [R, DA], dt.float32, kind="ExternalInput").ap()
    X = nc.dram_tensor("X", [T, D], dt.float32, kind="ExternalInput").ap()
    XL = nc.dram_tensor("XL", [R, D], dt.float32, kind="ExternalInput").ap()
    GAPS = nc.dram_tensor("GAPS", [R, N], dt.float32, kind="ExternalInput").ap()
    NCHF = nc.dram_tensor("NCHF", [R, N], dt.float32, kind="ExternalInput").ap()
    IOTA8 = nc.dram_tensor("IOTA8", [P, 8], dt.float32, kind="ExternalInput").ap()
    OUT = nc.dram_tensor("OUT", [R * N, D], dt.float32, kind="ExternalOutput").ap()
    OUT3 = OUT.rearrange("(r n) d -> r n d", n=N)

    with tile.TileContext(nc) as tc:
        with (
            tc.tile_pool(name="const", bufs=1) as const,
            tc.tile_pool(name="wk", bufs=2) as wk,
            tc.tile_pool(name="io", bufs=2) as io,
            tc.tile_pool(name="ps", bufs=2, space="PSUM") as ps,
        ):
            # ---- resident operands: X^T fp16 in 4x4 chunks, local 2X^T ----
            CCH = 2048
            NG = T // CCH
            xlt = [const.tile([P, R], mmdt, name=f"xlt{k}") for k in range(KC)]
            xt = [[const.tile([P, CCH], mmdt, name=f"xt{k}_{g}") for g in range(NG)]
                  for k in range(KC)]
            for k in range(KC):
                nc.sync.dma_start(xlt[k][:], XLT2H[k * P:(k + 1) * P, :])
            for k in range(KC):
                nc.sync.dma_start(xt[k][0][:], XTH[k * P:(k + 1) * P, 0:CCH])
            neg3 = const.tile([3, T], mmdt)
            ones3 = const.tile([3, P], mmdt)
            nc.sync.dma_start(neg3[:], NEG3[:])
            nc.sync.dma_start(ones3[:], ONES3[:])
            for g in range(1, NG):
                for k in range(KC):
                    nc.sync.dma_start(xt[k][g][:], XTH[k * P:(k + 1) * P, g * CCH:(g + 1) * CCH])

            iota8 = const.tile([P, 8], dt.float32)
            nc.sync.dma_start(iota8[:], IOTA8[:])

            def stage_a(rb):
                """GEMM -> cast -> top-8 -> launch candidate gathers + loads."""
                m0 = rb * P
                s16 = wk.tile([P, T], sdt, name=f"s16_{rb}", tag="s16", bufs=sbufs)
                for pg in range(NG):
                    pt = ps.tile([P, CCH], dt.float32, name=f"pt_{rb}_{pg}", tag="pt")
                    for k in range(KC):
                        for cbi in range(CCH // NB):
                            gb = cbi * NB
                            nc.tensor.matmul(pt[:, gb:gb + NB], lhsT=xlt[k][:, m0:m0 + P],
                                             rhs=xt[k][pg][:, gb:gb + NB],
                                             start=(k == 0), stop=False,
                                             skip_group_check=True)
                    for cbi in range(CCH // NB):
                        gb = cbi * NB
                        b0 = pg * CCH + gb
                        nc.tensor.matmul(pt[:, gb:gb + NB], lhsT=ones3[:, :],
                                         rhs=neg3[:, b0:b0 + NB], start=False, stop=True,
                                         skip_group_check=True)
                    nc.scalar.activation(s16[:, pg * CCH:(pg + 1) * CCH], pt[:],
                                         AF.Copy, bias=512.0, scale=1.0)

                vals8 = wk.tile([P, 8], sdt, name=f"v8_{rb}", tag="v8")
                idxu = wk.tile([P, 8], dt.uint32, name=f"iu_{rb}", tag="iu")
                nc.vector.max(out=vals8[:], in_=s16[:])
                nc.vector.max_index(out=idxu[:], in_max=vals8[:], in_values=s16[:])

                xg = io.tile([P, NCAND, DA], dt.float32, name=f"xg_{rb}", tag="xg")
                for j in range(NCAND):
                    nc.gpsimd.indirect_dma_start(
                        out=xg[:, j, :], out_offset=None, in_=XAUG[:],
                        in_offset=bass.IndirectOffsetOnAxis(ap=idxu[:, j + 1:j + 2], axis=0))
                xb2a = io.tile([P, DA], dt.float32, name=f"xb2a_{rb}", tag="xb2a")
                nc.sync.dma_start(xb2a[:], XB2A[m0:m0 + P, :])
                ncf = io.tile([P, N], dt.float32, name=f"ncf_{rb}", tag="ncf")
                nc.sync.dma_start(ncf[:], NCHF[m0:m0 + P, :])
                gaps_t = io.tile([P, N], dt.float32, name=f"gp_{rb}", tag="gp")
                nc.sync.dma_start(gaps_t[:], GAPS[m0:m0 + P, :])
                xb = io.tile([P, D], dt.float32, name=f"xb_{rb}", tag="xb")
                nc.sync.dma_start(xb[:], XL[m0:m0 + P, :])
                return dict(idxu=idxu, xg=xg, xb2a=xb2a, ncf=ncf, gaps_t=gaps_t, xb=xb, m0=m0)

            def stage_b(rb, st):
                """Re-rank -> map -> gather selected -> interpolate -> store."""
                idxu, xg, xb2a = st["idxu"], st["xg"], st["xb2a"]
                ncf, gaps_t, xb, m0 = st["ncf"], st["gaps_t"], st["xb"], st["m0"]
                # batched exact dot products: one wide mul, per-candidate ACT reduce
                scrB = wk.tile([P, NCAND, DA], dt.float32, name=f"scrB_{rb}", tag="scrB")
                nc.vector.tensor_mul(scrB[:, :, :], xg[:, :, :],
                                     xb2a[:, None, :].broadcast_to([P, NCAND, DA]))
                sex = wk.tile([P, 8], dt.float32, name=f"sex_{rb}", tag="sex")
                nc.vector.memset(sex[:], -3.0e38)
                for j in range(NCAND):
                    scr2 = wk.tile([P, DA], dt.float32, name=f"scr2_{rb}_{j}", tag="scr2")
                    nc.scalar.activation(scr2[:], scrB[:, j, :], AF.Copy,
                                         accum_out=sex[:, j:j + 1])

                # sort the exact values; p2[r, t] = slot (0..NCAND-1) of rank-t
                v2 = wk.tile([P, 8], dt.float32, name=f"v2_{rb}", tag="v2")
                p2u = wk.tile([P, 8], dt.uint32, name=f"p2u_{rb}", tag="p2u")
                p2f = wk.tile([P, 8], dt.float32, name=f"p2f_{rb}", tag="p2f")
                nc.vector.max(out=v2[:], in_=sex[:])
                nc.vector.max_index(out=p2u[:], in_max=v2[:], in_values=sex[:])
                nc.gpsimd.tensor_copy(p2f[:], p2u[:])
                gidxf = wk.tile([P, 8], dt.float32, name=f"gx_{rb}", tag="gx")
                nc.gpsimd.tensor_copy(gidxf[:], idxu[:])

                # fsel[r, n] = p2[r, nnc[r, n]] via broadcast one-hot + X-reduce
                q1 = wk.tile([P, N, N], dt.float32, name=f"q1_{rb}", tag="q1")
                nc.vector.tensor_tensor(q1[:, :, :],
                                        ncf[:, :, None].broadcast_to([P, N, N]),
                                        iota8[:, None, :N].broadcast_to([P, N, N]),
                                        ALU.is_equal)
                nc.vector.tensor_mul(q1[:, :, :], q1[:, :, :],
                                     p2f[:, None, :N].broadcast_to([P, N, N]))
                fsel = wk.tile([P, N], dt.float32, name=f"fs_{rb}", tag="fs")
                nc.vector.tensor_reduce(out=fsel[:, :], in_=q1[:, :, :],
                                        axis=mybir.AxisListType.X, op=ALU.add)
                # sel[r, n] = idxu[r, 1 + fsel[r, n]]
                q2 = wk.tile([P, N, NCAND], dt.float32, name=f"q2_{rb}", tag="q2")
                nc.vector.tensor_tensor(q2[:, :, :],
                                        fsel[:, :, None].broadcast_to([P, N, NCAND]),
                                        iota8[:, None, :NCAND].broadcast_to([P, N, NCAND]),
                                        ALU.is_equal)
                nc.vector.tensor_mul(q2[:, :, :], q2[:, :, :],
                                     gidxf[:, None, 1:1 + NCAND].broadcast_to([P, N, NCAND]))
                self_f = wk.tile([P, N], dt.float32, name=f"sf_{rb}", tag="sf")
                nc.vector.tensor_reduce(out=self_f[:, :], in_=q2[:, :, :],
                                        axis=mybir.AxisListType.X, op=ALU.add)
                selu = wk.tile([P, N], dt.uint32, name=f"su_{rb}", tag="su")
                nc.gpsimd.tensor_copy(selu[:], self_f[:])

                # interpolate: ot = g*xs + (1-g)*xb
                hfac = wk.tile([P, N], dt.float32, name=f"hf_{rb}", tag="hf")
                nc.gpsimd.tensor_scalar(out=hfac[:], in0=gaps_t[:], scalar1=-1.0,
                                        scalar2=1.0, op0=ALU.mult, op1=ALU.add)
                xs4 = io.tile([P, N, D], dt.float32, name=f"xs4_{rb}", tag="xs4")
                for n in range(N):
                    nc.gpsimd.indirect_dma_start(
                        out=xs4[:, n, :], out_offset=None, in_=X[:],
                        in_offset=bass.IndirectOffsetOnAxis(ap=selu[:, n:n + 1], axis=0))
                    ht = io.tile([P, D], dt.float32, name=f"ht_{rb}_{n}", tag="ht", bufs=2)
                    nc.scalar.activation(ht[:], xb[:], AF.Copy, scale=hfac[:, n:n + 1])
                    df = io.tile([P, D], dt.float32, name=f"df_{rb}_{n}", tag="df", bufs=2)
                    nc.scalar.activation(df[:], xs4[:, n, :], AF.Copy,
                                         scale=gaps_t[:, n:n + 1])
                    ot = io.tile([P, D], dt.float32, name=f"ot_{rb}_{n}", tag="ot", bufs=2)
                    nc.gpsimd.tensor_add(ot[:], df[:], ht[:])
                    nc.sync.dma_start(OUT3[m0:m0 + P, n, :], ot[:])

            # software-pipelined emission: A(rb+1) issued before B(rb) so each
            # engine can run the next block's front while this block's tail waits
            prev = stage_a(0)
            for rb in range(1, RB):
                cur = stage_a(rb)
                stage_b(rb - 1, prev)
                prev = cur
            stage_b(RB - 1, prev)

    nc.compile()
    return nc


def _bf16(x):
    import ml_dtypes
    return x.astype(ml_dtypes.bfloat16)


def _pair_round(x):
    hi = _bf16(x).astype(np.float32)
    lo = _bf16(x - hi).astype(np.float32)
    return hi + lo


V2DT = os.environ.get("SMOTE_V2_DT", "fp16")
V2TTR = os.environ.get("SMOTE_V2_TTR", "0") == "1"
V2SDT = os.environ.get("SMOTE_V2_SDT", "fp16")
V2MG = os.environ.get("SMOTE_V2_MG", "0") == "1"


def _get_nc(mode):
    key = (mode, V2DT, V2TTR, V2SDT, V2MG, os.environ.get("SMOTE_V2_NC", "5")) if mode == "v2" else mode
    if key not in _cache:
        _cache[key] = _build_v2(V2DT, V2TTR, V2SDT, V2MG) if mode == "v2" else _build(mode)
    return _cache[key]


def _kernel_v2(X, gaps, nnc):
    from concourse.bass_utils import run_bass_kernel_spmd

    nc = _get_nc("v2")

    sq = np.einsum("td,td->t", X, X, dtype=np.float32).astype(np.float32)
    negsq = -sq
    if V2DT == "fp16":
        f16 = lambda a: a.astype(np.float16)
    else:
        import ml_dtypes
        f16 = lambda a: a.astype(ml_dtypes.bfloat16)
    n1 = f16(negsq).astype(np.float32)
    n2 = f16(negsq - n1).astype(np.float32)
    n3 = f16(negsq - n1 - n2).astype(np.float32)
    NEG3 = np.ascontiguousarray(np.stack([f16(n1), f16(n2), f16(n3)]))
    ONES3 = np.ascontiguousarray(f16(np.ones((3, P), dtype=np.float32)))
    XTH = np.ascontiguousarray(f16(X.T))
    xaug = np.zeros((T, DA), dtype=np.float32)
    xaug[:, :D] = X
    xaug[:, D] = negsq
    iota8 = np.broadcast_to(np.arange(8, dtype=np.float32)[None, :], (P, 8)).copy()
    common = dict(XTH=XTH, NEG3=NEG3, ONES3=ONES3, XAUG=xaug, X=X, IOTA8=iota8)

    in_maps = []
    for c in range(NCORES):
        r0 = c * R
        xl = X[r0:r0 + R]
        m = dict(common)
        m["XLT2H"] = np.ascontiguousarray(f16((2.0 * xl).T))
        m["XL"] = np.ascontiguousarray(xl)
        xb2a = np.zeros((R, DA), dtype=np.float32)
        xb2a[:, :D] = 2.0 * xl
        xb2a[:, D] = 1.0
        m["XB2A"] = xb2a
        m["GAPS"] = np.ascontiguousarray(gaps[r0:r0 + R])
        m["NCHF"] = np.ascontiguousarray(nnc[r0:r0 + R].astype(np.float32))
        in_maps.append(m)
    return nc, in_maps


def kernel(X, gaps, nn_choice, k, _want_results=False, _trace=False):
    X = np.ascontiguousarray(np.asarray(X, dtype=np.float32))
    gaps = np.ascontiguousarray(np.asarray(gaps, dtype=np.float32))
    nnc = np.asarray(nn_choice).astype(np.int64)
    assert int(k) == KNN and X.shape == (T, D) and gaps.shape == (T, N)

    from concourse.bass_utils import run_bass_kernel_spmd

    mode = MODE
    if mode == "v2":
        nc, in_maps = _kernel_v2(X, gaps, nnc)
        res = run_bass_kernel_spmd(nc, in_maps, core_ids=list(range(NCORES)), trace=_trace)
        out = np.concatenate([res.results[c]["OUT"] for c in range(NCORES)], axis=0)
        if _want_results:
            return out, res
        return out
    nc = _get_nc(mode)

    sq = np.einsum("td,td->t", X, X, dtype=np.float32).astype(np.float32)
    negsq = -sq
    n1 = _bf16(negsq).astype(np.float32)
    n2 = _bf16(negsq - n1).astype(np.float32)
    n3 = _bf16(negsq - n1 - n2).astype(np.float32)
    NEG3 = np.ascontiguousarray(np.stack([_bf16(n1), _bf16(n2), _bf16(n3)]))
    ONES3 = np.ascontiguousarray(np.ones((3, P), dtype=np.float32).astype(NEG3.dtype))
    XTc = np.ascontiguousarray(X.T)

    common = dict(NEG3=NEG3, ONES3=ONES3, X=X)
    if mode == "fp32r_rr":
        xaug = np.zeros((T, DA), dtype=np.float32)
        xaug[:, :D] = X
        xaug[:, D] = negsq
        common["XAUG"] = xaug
    if mode in ("fp32r", "fp32r_rr"):
        common["XT"] = np.ascontiguousarray(_pair_round(XTc))
    elif mode == "fp32":
        common["XT"] = XTc
    else:
        xth = _bf16(XTc)
        common["XTH"] = np.ascontiguousarray(xth)
        common["XTL"] = np.ascontiguousarray(_bf16(XTc - xth.astype(np.float32)))

    in_maps = []
    for c in range(NCORES):
        r0 = c * R
        xl = X[r0:r0 + R]
        xlt2 = np.ascontiguousarray((2.0 * xl).T)
        m = dict(common)
        if mode in ("fp32r", "fp32r_rr"):
            m["XLT2"] = np.ascontiguousarray(_pair_round(xlt2))
        elif mode == "fp32":
            m["XLT2"] = xlt2
        else:
            h = _bf16(xlt2)
            m["XLT2H"] = np.ascontiguousarray(h)
            m["XLT2L"] = np.ascontiguousarray(_bf16(xlt2 - h.astype(np.float32)))
        m["XL"] = np.ascontiguousarray(xl)
        if mode == "fp32r_rr":
            xb2a = np.zeros((R, DA), dtype=np.float32)
            xb2a[:, :D] = 2.0 * xl
            xb2a[:, D] = 1.0
            m["XB2A"] = xb2a
        m["GAPS"] = np.ascontiguousarray(gaps[r0:r0 + R])
        m["NCHF"] = np.ascontiguousarray(nnc[r0:r0 + R].astype(np.float32))
        in_maps.append(m)

    res = run_bass_kernel_spmd(nc, in_maps, core_ids=list(range(NCORES)), trace=_trace)
    out = np.concatenate([res.results[c]["OUT"] for c in range(NCORES)], axis=0)
    if _want_results:
        return out, res
    return out



# revision 14
# speedup vs baseline: 1.4063x; 1.0443x over previous
"""SMOTE.generate kernel for 8 TRN2 NeuronCores (Bass/Tile).

Problem: X [8192, 512] f32 -> pairwise sq-dists -> per-row 4 nearest
non-self neighbors -> pick by nn_choice -> synth = X + gaps*(X[sel]-X).
Output [32768, 512] f32.

Strategy (data-parallel over rows, 1024 rows/core):
  - s[r, c] = 2*x_r . x_c - |x_c|^2  has the same per-row ordering as
    -dist (per-row constant |x_r|^2 dropped; sqrt monotone).  Self is
    always the row max (|x_r|^2 vs ~ -|x_c|^2), matching the reference's
    top-1-is-self behavior.
  - GEMM on TensorE in fp32r (bf16-pair datapath, 4x faster than fp32) or
    bf16x3 (exact hi/lo split) / fp32 fallbacks; -|x_c|^2 enters as a
    rank-3 bf16 matmul (ones x [hi;lo;lo2] split of -sq).
  - Per 128-row block: DVE max8 + find_index8 over each 4096-col half,
    merge the 16 candidates, one-hot select by nn_choice, indirect-DMA
    gather X[sel], interpolate exactly in fp32.
"""
import os
import sys

import numpy as np

sys.path.insert(0, "/opt/trn_rl_repo")

T, D, N, KNN = 8192, 512, 4, 5
NCORES = 8
R = T // NCORES          # 1024 rows per core
P = 128
RB = R // P              # 8 row blocks per core
HALVES = 2
CH = T // HALVES         # 4096 columns per half
NB = 512                 # matmul free dim (one PSUM bank of fp32)
CB = CH // NB            # 8 col blocks per half
KC = D // P              # 4 contraction chunks of 128
DA = 528                 # gather row: x (512) | -sq (1) | pad; 64B-aligned rows

MODE = os.environ.get("SMOTE_MODE", "v2")  # v2 | bf16x3 | fp32r | fp32r_rr | fp32

_cache = {}


def _build_v2(v2dt="bf16", use_ttr=True, v2sdt="bf16", multigather=False):
    """Single-pass low-precision GEMM shortlist + exact fp32 re-rank.

    s = 2*x_r.x_c - |x_c|^2 computed once in fp16 (1 cyc/row on PE, 3x
    cheaper than bf16x3).  PSUM is cast-copied to fp16 SBUF (+512 shift
    keeps values small for finer quantization).  DVE max8/find_index8
    gives an 8-wide shortlist per row (slot 0 is always self).  The 7
    non-self candidates are gathered in fp32 and re-ranked exactly with
    fused mul+reduce dot products, which restores the reference's fp32
    ordering (host sim: 0/32768 rows differ).
    """
    import concourse.bass as bass
    import concourse.bacc as bacc
    import concourse.mybir as mybir
    import concourse.tile as tile

    dt = mybir.dt
    AF = mybir.ActivationFunctionType
    ALU = mybir.AluOpType
    nc = bacc.Bacc("TRN2", target_bir_lowering=False, debug=False)

    mmdt = dt.float16 if v2dt == "fp16" else dt.bfloat16
    sdt = {"fp16": dt.float16, "bf16": dt.bfloat16, "fp32": dt.float32}[v2sdt]
    sbufs = 1 if v2sdt == "fp32" else 2  # fp32 s is 32KB/partition
    NCAND = int(os.environ.get("SMOTE_V2_NC", "5"))  # non-self shortlist slots

    XTH = nc.dram_tensor("XTH", [D, T], mmdt, kind="ExternalInput").ap()
    XLT2H = nc.dram_tensor("XLT2H", [D, R], mmdt, kind="ExternalInput").ap()
    NEG3 = nc.dram_tensor("NEG3", [3, T], mmdt, kind="ExternalInput").ap()
    ONES3 = nc.dram_tensor("ONES3", [3, P], mmdt, kind="ExternalInput").ap()
    XAUG = nc.dram_tensor("XAUG", [T, DA], dt.float32, kind="ExternalInput").ap()
    XB2A = nc.dram_tensor("XB2A", [R, DA], dt.float32, kind="ExternalInput").ap()
    X = nc.dram_tensor("X", [T, D], dt.float32, kind="ExternalInput").ap()
    XL = nc.dram_tensor("XL", [R, D], dt.float32, kind="ExternalInput").ap()
    GAPS = nc.dram_tensor("GAPS", [R, N], dt.float32, kind="ExternalInput").ap()
    NCHF = nc.dram_tensor("NCHF", [R, N], dt.float32, kind="ExternalInput").ap()
    IOTA8 = nc.dram_tensor("IOTA8", [P, 8], dt.float32, kind="ExternalInput").ap()
    OUT = nc.dram_tensor("OUT", [R * N, D], dt.float32, kind="ExternalOutput").ap()
    OUT3 = OUT.rearrange("(r n) d -> r n d", n=N)

    with tile.TileContext(nc) as tc:
        with (
            tc.tile_pool(name="const", bufs=1) as const,
            tc.tile_pool(name="wk", bufs=2) as wk,
            tc.tile_pool(name="io", bufs=2) as io,
            tc.tile_pool(name="ps", bufs=2, space="PSUM") as ps,
        ):
            # ---- resident operands: X^T fp16 in 4x4 chunks, local 2X^T ----
            CCH = 2048
            NG = T // CCH
            xlt = [const.tile([P, R], mmdt, name=f"xlt{k}") for k in range(KC)]
            xt = [[const.tile([P, CCH], mmdt, name=f"xt{k}_{g}") for g in range(NG)]
                  for k in range(KC)]
            for k in range(KC):
                nc.sync.dma_start(xlt[k][:], XLT2H[k * P:(k + 1) * P, :])
            for k in range(KC):
                nc.sync.dma_start(xt[k][0][:], XTH[k * P:(k + 1) * P, 0:CCH])
            neg3 = const.tile([3, T], mmdt)
            ones3 = const.tile([3, P], mmdt)
            nc.sync.dma_start(neg3[:], NEG3[:])
            nc.sync.dma_start(ones3[:], ONES3[:])
            for g in range(1, NG):
                for k in range(KC):
                    nc.sync.dma_start(xt[k][g][:], XTH[k * P:(k + 1) * P, g * CCH:(g + 1) * CCH])

            iota8 = const.tile([P, 8], dt.float32)
            nc.sync.dma_start(iota8[:], IOTA8[:])

            def stage_a(rb):
                """GEMM -> cast -> top-8 -> launch candidate gathers + loads."""
                m0 = rb * P
                s16 = wk.tile([P, T], sdt, name=f"s16_{rb}", tag="s16", bufs=sbufs)
                for pg in range(NG):
                    pt = ps.tile([P, CCH], dt.float32, name=f"pt_{rb}_{pg}", tag="pt")
                    for k in range(KC):
                        for cbi in range(CCH // NB):
                            gb = cbi * NB
                            nc.tensor.matmul(pt[:, gb:gb + NB], lhsT=xlt[k][:, m0:m0 + P],
                                             rhs=xt[k][pg][:, gb:gb + NB],
                                             start=(k == 0), stop=False,
                                             skip_group_check=True)
                    for cbi in range(CCH // NB):
                        gb = cbi * NB
                        b0 = pg * CCH + gb
                        nc.tensor.matmul(pt[:, gb:gb + NB], lhsT=ones3[:, :],
                                         rhs=neg3[:, b0:b0 + NB], start=False, stop=True,
                                         skip_group_check=True)
                    nc.scalar.activation(s16[:, pg * CCH:(pg + 1) * CCH], pt[:],
                                         AF.Copy, bias=512.0, scale=1.0)

                vals8 = wk.tile([P, 8], sdt, name=f"v8_{rb}", tag="v8")
                idxu = wk.tile([P, 8], dt.uint32, name=f"iu_{rb}", tag="iu")
                nc.vector.max(out=vals8[:], in_=s16[:])
                nc.vector.max_index(out=idxu[:], in_max=vals8[:], in_values=s16[:])

                xg = io.tile([P, NCAND, DA], dt.float32, name=f"xg_{rb}", tag="xg")
                for j in range(NCAND):
                    nc.gpsimd.indirect_dma_start(
                        out=xg[:, j, :], out_offset=None, in_=XAUG[:],
                        in_offset=bass.IndirectOffsetOnAxis(ap=idxu[:, j + 1:j + 2], axis=0))
                gidxf = wk.tile([P, 8], dt.float32, name=f"gx_{rb}", tag="gx")
                nc.gpsimd.tensor_copy(gidxf[:], idxu[:])
                xb2a = io.tile([P, DA], dt.float32, name=f"xb2a_{rb}", tag="xb2a")
                nc.sync.dma_start(xb2a[:], XB2A[m0:m0 + P, :])
                ncf = io.tile([P, N], dt.float32, name=f"ncf_{rb}", tag="ncf")
                nc.sync.dma_start(ncf[:], NCHF[m0:m0 + P, :])
                gaps_t = io.tile([P, N], dt.float32, name=f"gp_{rb}", tag="gp")
                nc.sync.dma_start(gaps_t[:], GAPS[m0:m0 + P, :])
                xb = io.tile([P, D], dt.float32, name=f"xb_{rb}", tag="xb")
                nc.sync.dma_start(xb[:], XL[m0:m0 + P, :])
                return dict(idxu=idxu, xg=xg, xb2a=xb2a, ncf=ncf, gaps_t=gaps_t,
                            xb=xb, m0=m0, gidxf=gidxf)

            def stage_b(rb, st):
                """Re-rank -> map -> gather selected -> interpolate -> store."""
                idxu, xg, xb2a = st["idxu"], st["xg"], st["xb2a"]
                ncf, gaps_t, xb, m0 = st["ncf"], st["gaps_t"], st["xb"], st["m0"]
                gidxf = st["gidxf"]
                # batched exact dot products: one wide mul, per-candidate ACT reduce
                scrB = wk.tile([P, NCAND, DA], dt.float32, name=f"scrB_{rb}", tag="scrB")
                nc.vector.tensor_mul(scrB[:, :, :], xg[:, :, :],
                                     xb2a[:, None, :].broadcast_to([P, NCAND, DA]))
                sex = wk.tile([P, 8], dt.float32, name=f"sex_{rb}", tag="sex")
                for j in range(NCAND):
                    scr2 = wk.tile([P, DA], dt.float32, name=f"scr2_{rb}_{j}", tag="scr2")
                    nc.scalar.activation(scr2[:], scrB[:, j, :], AF.Copy,
                                         accum_out=sex[:, j:j + 1])

                # rank each candidate by pairwise compares (no sort needed):
                # rank[j] = #{j': sex[j'] > sex[j]};  sel[r,n] = gidx[1+j] where
                # rank[j] == nnc[r,n]
                q3 = wk.tile([P, NCAND, NCAND], dt.float32, name=f"q3_{rb}", tag="q3")
                nc.vector.tensor_tensor(q3[:, :, :],
                                        sex[:, None, :NCAND].broadcast_to([P, NCAND, NCAND]),
                                        sex[:, :NCAND, None].broadcast_to([P, NCAND, NCAND]),
                                        ALU.is_gt)
                rank = wk.tile([P, NCAND], dt.float32, name=f"rk_{rb}", tag="rk")
                nc.vector.tensor_reduce(out=rank[:, :], in_=q3[:, :, :],
                                        axis=mybir.AxisListType.X, op=ALU.add)
                q4 = wk.tile([P, N, NCAND], dt.float32, name=f"q4_{rb}", tag="q4")
                nc.vector.tensor_tensor(q4[:, :, :],
                                        rank[:, None, :].broadcast_to([P, N, NCAND]),
                                        ncf[:, :, None].broadcast_to([P, N, NCAND]),
                                        ALU.is_equal)
                nc.vector.tensor_mul(q4[:, :, :], q4[:, :, :],
                                     gidxf[:, None, 1:1 + NCAND].broadcast_to([P, N, NCAND]))
                self_f = wk.tile([P, N], dt.float32, name=f"sf_{rb}", tag="sf")
                nc.vector.tensor_reduce(out=self_f[:, :], in_=q4[:, :, :],
                                        axis=mybir.AxisListType.X, op=ALU.add)
                selu = wk.tile([P, N], dt.uint32, name=f"su_{rb}", tag="su")
                nc.gpsimd.tensor_copy(selu[:], self_f[:])

                # interpolate: ot = g*xs + (1-g)*xb
                hfac = wk.tile([P, N], dt.float32, name=f"hf_{rb}", tag="hf")
                nc.gpsimd.tensor_scalar(out=hfac[:], in0=gaps_t[:], scalar1=-1.0,
                                        scalar2=1.0, op0=ALU.mult, op1=ALU.add)
                xs4 = io.tile([P, N, D], dt.float32, name=f"xs4_{rb}", tag="xs4")
                for n in range(N):
                    nc.gpsimd.indirect_dma_start(
                        out=xs4[:, n, :], out_offset=None, in_=X[:],
                        in_offset=bass.IndirectOffsetOnAxis(ap=selu[:, n:n + 1], axis=0))
                    ht = io.tile([P, D], dt.float32, name=f"ht_{rb}_{n}", tag="ht", bufs=2)
                    nc.scalar.activation(ht[:], xb[:], AF.Copy, scale=hfac[:, n:n + 1])
                    df = io.tile([P, D], dt.float32, name=f"df_{rb}_{n}", tag="df", bufs=2)
                    nc.scalar.activation(df[:], xs4[:, n, :], AF.Copy,
                                         scale=gaps_t[:, n:n + 1])
                    ot = io.tile([P, D], dt.float32, name=f"ot_{rb}_{n}", tag="ot", bufs=2)
                    nc.gpsimd.tensor_add(ot[:], df[:], ht[:])
                    nc.sync.dma_start(OUT3[m0:m0 + P, n, :], ot[:])

            # software-pipelined emission: A(rb+1) issued before B(rb) so each
            # engine can run the next block's front while this block's tail waits
            prev = stage_a(0)
            for rb in range(1, RB):
                cur = stage_a(rb)
                stage_b(rb - 1, prev)
                prev = cur
            stage_b(RB - 1, prev)

    nc.compile()
    return nc


def _bf16(x):
    import ml_dtypes
    return x.astype(ml_dtypes.bfloat16)


def _pair_round(x):
    hi = _bf16(x).astype(np.float32)
    lo = _bf16(x - hi).astype(np.float32)
    return hi + lo


V2DT = os.environ.get("SMOTE_V2_DT", "fp16")
V2TTR = os.environ.get("SMOTE_V2_TTR", "0") == "1"
V2SDT = os.environ.get("SMOTE_V2_SDT", "fp16")
V2MG = os.environ.get("SMOTE_V2_MG", "0") == "1"


def _get_nc(mode):
    key = (mode, V2DT, V2TTR, V2SDT, V2MG, os.environ.get("SMOTE_V2_NC", "5")) if mode == "v2" else mode
    if key not in _cache:
        _cache[key] = _build_v2(V2DT, V2TTR, V2SDT, V2MG) if mode == "v2" else _build(mode)
    return _cache[key]


def _kernel_v2(X, gaps, nnc):
    from concourse.bass_utils import run_bass_kernel_spmd

    nc = _get_nc("v2")

    sq = np.einsum("td,td->t", X, X, dtype=np.float32).astype(np.float32)
    negsq = -sq
    if V2DT == "fp16":
        f16 = lambda a: a.astype(np.float16)
    else:
        import ml_dtypes
        f16 = lambda a: a.astype(ml_dtypes.bfloat16)
    n1 = f16(negsq).astype(np.float32)
    n2 = f16(negsq - n1).astype(np.float32)
    n3 = f16(negsq - n1 - n2).astype(np.float32)
    NEG3 = np.ascontiguousarray(np.stack([f16(n1), f16(n2), f16(n3)]))
    ONES3 = np.ascontiguousarray(f16(np.ones((3, P), dtype=np.float32)))
    XTH = np.ascontiguousarray(f16(X.T))
    xaug = np.zeros((T, DA), dtype=np.float32)
    xaug[:, :D] = X
    xaug[:, D] = negsq
    iota8 = np.broadcast_to(np.arange(8, dtype=np.float32)[None, :], (P, 8)).copy()
    common = dict(XTH=XTH, NEG3=NEG3, ONES3=ONES3, XAUG=xaug, X=X, IOTA8=iota8)

    in_maps = []
    for c in range(NCORES):
        r0 = c * R
        xl = X[r0:r0 + R]
        m = dict(common)
        m["XLT2H"] = np.ascontiguousarray(f16((2.0 * xl).T))
        m["XL"] = np.ascontiguousarray(xl)
        xb2a = np.zeros((R, DA), dtype=np.float32)
        xb2a[:, :D] = 2.0 * xl
        xb2a[:, D] = 1.0
        m["XB2A"] = xb2a
        m["GAPS"] = np.ascontiguousarray(gaps[r0:r0 + R])
        m["NCHF"] = np.ascontiguousarray(nnc[r0:r0 + R].astype(np.float32))
        in_maps.append(m)
    return nc, in_maps


def kernel(X, gaps, nn_choice, k, _want_results=False, _trace=False):
    X = np.ascontiguousarray(np.asarray(X, dtype=np.float32))
    gaps = np.ascontiguousarray(np.asarray(gaps, dtype=np.float32))
    nnc = np.asarray(nn_choice).astype(np.int64)
    assert int(k) == KNN and X.shape == (T, D) and gaps.shape == (T, N)

    from concourse.bass_utils import run_bass_kernel_spmd

    mode = MODE
    if mode == "v2":
        nc, in_maps = _kernel_v2(X, gaps, nnc)
        res = run_bass_kernel_spmd(nc, in_maps, core_ids=list(range(NCORES)), trace=_trace)
        out = np.concatenate([res.results[c]["OUT"] for c in range(NCORES)], axis=0)
        if _want_results:
            return out, res
        return out
    nc = _get_nc(mode)

    sq = np.einsum("td,td->t", X, X, dtype=np.float32).astype(np.float32)
    negsq = -sq
    n1 = _bf16(negsq).astype(np.float32)
    n2 = _bf16(negsq - n1).astype(np.float32)
    n3 = _bf16(negsq - n1 - n2).astype(np.float32)
    NEG3 = np.ascontiguousarray(np.stack([_bf16(n1), _bf16(n2), _bf16(n3)]))
    ONES3 = np.ascontiguousarray(np.ones((3, P), dtype=np.float32).astype(NEG3.dtype))
    XTc = np.ascontiguousarray(X.T)

    common = dict(NEG3=NEG3, ONES3=ONES3, X=X)
    if mode == "fp32r_rr":
        xaug = np.zeros((T, DA), dtype=np.float32)
        xaug[:, :D] = X
        xaug[:, D] = negsq
        common["XAUG"] = xaug
    if mode in ("fp32r", "fp32r_rr"):
        common["XT"] = np.ascontiguousarray(_pair_round(XTc))
    elif mode == "fp32":
        common["XT"] = XTc
    else:
        xth = _bf16(XTc)
        common["XTH"] = np.ascontiguousarray(xth)
        common["XTL"] = np.ascontiguousarray(_bf16(XTc - xth.astype(np.float32)))

    in_maps = []
    for c in range(NCORES):
        r0 = c * R
        xl = X[r0:r0 + R]
        xlt2 = np.ascontiguousarray((2.0 * xl).T)
        m = dict(common)
        if mode in ("fp32r", "fp32r_rr"):
            m["XLT2"] = np.ascontiguousarray(_pair_round(xlt2))
        elif mode == "fp32":
            m["XLT2"] = xlt2
        else:
            h = _bf16(xlt2)
            m["XLT2H"] = np.ascontiguousarray(h)
            m["XLT2L"] = np.ascontiguousarray(_bf16(xlt2 - h.astype(np.float32)))
        m["XL"] = np.ascontiguousarray(xl)
        if mode == "fp32r_rr":
            xb2a = np.zeros((R, DA), dtype=np.float32)
            xb2a[:, :D] = 2.0 * xl
            xb2a[:, D] = 1.0
            m["XB2A"] = xb2a
        m["GAPS"] = np.ascontiguousarray(gaps[r0:r0 + R])
        m["NCHF"] = np.ascontiguousarray(nnc[r0:r0 + R].astype(np.float32))
        in_maps.append(m)

    res = run_bass_kernel_spmd(nc, in_maps, core_ids=list(range(NCORES)), trace=_trace)
    out = np.concatenate([res.results[c]["OUT"] for c in range(NCORES)], axis=0)
    if _want_results:
        return out, res
    return out

